# revision 14
# baseline (speedup 1.0000x reference)
"""Trainium2 Bass kernel for nn_MultiHeadAttention_45672682226228.

The reference module computes multi-head attention but everything except the
V projection is dead code (DCE'd under jit): the returned value is

    out[b, s, 64*h + q] = x[b, s, 768 + 64*h + q]
                        + sum_d x[b, s, 256*h + d] * W_v[q, d]

i.e. a per-token block-diagonal matmul (4 heads x [256 -> 64]) plus a
residual add of the last head's input slice.  W_q / W_k are unused.

Sharding: data-parallel over batch B=16 -> 2 batches (8192 tokens) per core
across 8 NeuronCores.  Inputs are shipped as bf16 (the 2e-2 rel-err budget
dwarfs bf16's ~4e-3 rounding), halving HBM read traffic, and pre-transposed
per shard so the contraction dim lands on SBUF partitions without any
on-chip transpose work.  The DRAM image IS the sequence of SBUF tiles
(xt[g, p, j, t] = x[512g + t, 128j + p]) so every partition line of a group
DMA is one contiguous 8 KiB run - big descriptors, line-rate HBM.  Per core:

  xt [16, 128, 8, 512] bf16  ->  out [16, 128, 2, 512] fp32
  (out[g, c, cc, t] = result[512g + t, 128cc + c]; host inverts both
  permutations during shard/gather)

On-chip dataflow per 2048-token super-group (4 of them, 4 sub-groups each):
  1. Four 1 MiB DMAs load the group tiles xT [128d, 8j, 512t] bf16,
     alternating the two HWDGE rings.
  2. TensorE, j-outer over all 8 PSUM banks: for each W_j (loaded once),
     4 accumulating bf16 matmuls (N=512), one per sub-group.  The j-outer
     order makes one dense ~8 us PE burst per super-group (HAM stays warm)
     and amortizes LDWEIGHTS 4x.
  3. DVE adds the residual from xT chunks 6/7 (= x[:, 768:1024].T, already
     on-chip) into the PSUM banks as they complete (cc=0 adds overlap the
     cc=1 matmuls), writing fp32 outT to SBUF.
  4. SWDGE DMA stores [128, 2, 512] result tiles (4 KiB partition lines).
"""

import os
import numpy as np

P = 128
TPC = 8192          # tokens per core
NCORES = 8
GT = 512            # tokens per group
GROUPS = TPC // GT  # 16

_STATE = {}


def _pack_wblk(W_v: np.ndarray) -> np.ndarray:
    """Pack W_v [64, 256] into per-d-chunk stationary blocks [128, 8, 128].

    wblk[dd, j, col]: d-chunk j covers global d in [128j, 128j+128);
    head h = j//2, half = j%2.  Within c-chunk cc = j//4 the head's 64
    output cols sit at offset 64*(h%2).  Zeros elsewhere.
    """
    W_v = np.asarray(W_v, np.float32)
    wblk = np.zeros((P, 8, P), np.float32)
    for j in range(8):
        h, half = j // 2, j % 2
        c0 = 64 * (h % 2)
        wblk[:, j, c0:c0 + 64] = W_v[:, 128 * half:128 * half + 128].T
    return wblk


def _build_nc(tpc=TPC):
    from contextlib import ExitStack

    import concourse.mybir as mybir
    import concourse.tile as tile
    from concourse import bacc

    f32 = mybir.dt.float32
    bf16 = mybir.dt.bfloat16
    groups = tpc // GT

    nc = bacc.Bacc("TRN2", target_bir_lowering=False, debug=False)
    xt_h = nc.dram_tensor("xt", [groups, P, 8, GT], bf16, kind="ExternalInput")
    w_h = nc.dram_tensor("wblk", [P, 8, P], bf16, kind="ExternalInput")
    o_h = nc.dram_tensor("out", [groups, P, 2, GT], f32, kind="ExternalOutput")

    SG = 4                       # sub-groups per super-group
    supers = groups // SG

    with ExitStack() as ctx:
        tc = ctx.enter_context(tile.TileContext(nc))
        const = ctx.enter_context(tc.tile_pool(name="const", bufs=1))
        xtp = ctx.enter_context(tc.tile_pool(name="xtp", bufs=3))
        osb = ctx.enter_context(tc.tile_pool(name="osb", bufs=3))
        ps_mm = ctx.enter_context(tc.tile_pool(name="ps_mm", bufs=1, space="PSUM"))

        w_sb = const.tile([P, 8, P], bf16)
        nc.sync.dma_start(w_sb[:], w_h[:])

        for sg in range(supers):
            xt_sb = xtp.tile([P, SG, 8, GT], bf16)
            for g in range(SG):
                # alternate the two HWDGE rings (SP / ACT)
                eng = nc.sync if g % 2 == 0 else nc.scalar
                eng.dma_start(xt_sb[:, g], xt_h[SG * sg + g])

            # j-outer: one LDWEIGHTS per W_j feeds 4 matmuls; all 8 PSUM
            # banks in flight -> one dense PE burst per super-group
            pm = ps_mm.tile([P, 8, GT], f32)
            for j in range(8):
                cc = j // 4
                for g in range(SG):
                    nc.tensor.matmul(
                        pm[:, 2 * g + cc, :],
                        w_sb[:, j, :],
                        xt_sb[:, g, j, :],
                        start=(j % 4 == 0),
                        stop=(j % 4 == 3),
                    )

            # residual: x[:, 768:1024].T lives in xT chunks 6/7.  cc=0 adds
            # unblock after j=3 and overlap the cc=1 matmuls.
            o_sb = osb.tile([P, SG, 2, GT], f32)
            for g in range(SG):
                nc.vector.tensor_add(
                    o_sb[:, g, 0, :], pm[:, 2 * g, :], xt_sb[:, g, 6, :]
                )
            for g in range(SG):
                nc.vector.tensor_add(
                    o_sb[:, g, 1, :], pm[:, 2 * g + 1, :], xt_sb[:, g, 7, :]
                )
                # SWDGE (GpSimd) so output stores don't head-of-line block
                # the input loads on the HWDGE rings
                nc.gpsimd.dma_start(o_h[SG * sg + g], o_sb[:, g])

    nc.compile()
    return nc


def _install_ntff_hook():
    """Provide antenv.axon_hooks (absent in this image) so trace=True works.

    Reconstructs the hook trn_boot would have registered at agent boot.
    """
    import sys
    import types

    if "antenv.axon_hooks" in sys.modules:
        return
    try:
        import trn_agent_boot.trn_boot as tb

        hook = tb._ntff_profile_via_ctypes("/opt/axon/libaxon_pjrt.so")
    except Exception:
        hook = None
    mod = types.ModuleType("antenv.axon_hooks")
    mod.get_axon_ntff_profile_hook = lambda: hook
    mod.set_axon_ntff_profile_hook = lambda h: None
    sys.modules["antenv.axon_hooks"] = mod
    try:
        import antenv

        antenv.axon_hooks = mod
    except ImportError:
        pass


def kernel(x, W_q=None, W_k=None, W_v=None, **_):
    import ml_dtypes

    from concourse.bass_utils import run_bass_kernel_spmd

    if "nc" not in _STATE:
        _STATE["nc"] = _build_nc()
    nc = _STATE["nc"]

    x = np.asarray(x, np.float32)
    b, s, e = x.shape
    # xb[c, g, t, j, p] = x[core c's token 512g+t, 128j+p], cast to bf16
    xb = x.reshape(NCORES, GROUPS, GT, 8, P).astype(ml_dtypes.bfloat16)
    wblk = _pack_wblk(W_v).astype(ml_dtypes.bfloat16)

    in_maps = [
        # -> xt[g, p, j, t]
        {"xt": np.ascontiguousarray(xb[c].transpose(0, 3, 2, 1)), "wblk": wblk}
        for c in range(NCORES)
    ]
    trace = os.environ.get("KERNEL_TRACE", "0") == "1"
    if trace:
        _install_ntff_hook()
    res = run_bass_kernel_spmd(nc, in_maps, core_ids=list(range(NCORES)), trace=trace)
    _STATE["last_results"] = res
    out = np.empty((NCORES, GROUPS, GT, 2, P), np.float32)
    for c in range(NCORES):
        # out[g, c', cc, t] -> [g, t, cc, c']
        np.copyto(out[c], res.results[c]["out"].transpose(0, 3, 2, 1))
    return out.reshape(b, s, 256)
